# revision 4
# baseline (speedup 1.0000x reference)
"""Chamfer-with-normals (6D NN search) Trainium2 kernel, v3.

Device program (per core, SPMD over 8 cores, no collectives):
  - 8 jobs = (batch b in 0..3) x (direction in {1,2}); core = 2*b + dir.
  - q[i,j] = 2*q6_i.db6_j - |db6_j|^2 via K=7 fp16 PE matmuls (f32 psum),
    DVE copy to SBUF, DVE max/max_index -> top-1 db index per query row.
  - Output: [128, 64] uint16 index matrix (16 KB) per core.

Host/runner:
  - ships one fp16 [7, n+m] matrix per core; exact fp32 metric on host.
  - per-core shards are device_put from 8 threads (axon RPCs overlap when
    issued concurrently; serial shard uploads dominate the wall otherwise),
    then assembled with make_array_from_single_device_arrays.
  - the donated output buffer is the previous call's device-resident output
    (first call uploads zeros once); output shards are fetched in threads.
"""

import sys
from concurrent.futures import ThreadPoolExecutor

import numpy as np

for _p in ("/opt/trn_rl_repo", "/opt/pypackages"):
    if _p not in sys.path:
        sys.path.insert(0, _p)

B = 4
N = 8192  # queries per job
M = 8192  # database per job
P = 128
EPS = 1e-12

_PROG_CACHE = {}


def _build_program(n, m, nb_limit=None):
    import concourse.bass as bass
    import concourse.tile as tile
    from concourse import mybir

    f16 = mybir.dt.float16
    f32 = mybir.dt.float32
    u16 = mybir.dt.uint16
    nb = nb_limit if nb_limit is not None else n // P  # query row blocks
    K = 7

    nc = bass.Bass()
    ab_d = nc.dram_tensor("ab", [K, n + m], f16, kind="ExternalInput")
    idx_d = nc.dram_tensor("idx", [P, nb], u16, kind="ExternalOutput")

    with tile.TileContext(nc) as tc:
        with (
            tc.tile_pool(name="singles", bufs=1) as singles,
            tc.tile_pool(name="qrows", bufs=2) as qrows,
            tc.tile_pool(name="tops", bufs=4) as tops,
            tc.tile_pool(name="qps", bufs=2, space="PSUM") as qps,
        ):
            ab_sb = singles.tile([K, n + m], f16)
            idx_sb = singles.tile([P, nb], u16)
            nc.sync.dma_start(out=ab_sb[:], in_=ab_d[:])

            for ib in range(nb):
                qrow = qrows.tile([P, m], f32)
                for rnd in range(m // 2048):
                    q = qps.tile([P, 2048], f32, space="PSUM")
                    for r in range(4):
                        c = rnd * 4 + r
                        nc.tensor.matmul(
                            out=q[:, r * 512 : (r + 1) * 512],
                            lhsT=ab_sb[:, ib * P : (ib + 1) * P],
                            rhs=ab_sb[:, n + c * 512 : n + (c + 1) * 512],
                            start=True,
                            stop=True,
                        )
                    nc.vector.tensor_copy(
                        qrow[:, rnd * 2048 : (rnd + 1) * 2048], q[:]
                    )
                top_val = tops.tile([P, 8], f32)
                top_idx = tops.tile([P, 8], u16)
                nc.vector.max(top_val[:], qrow[:])
                nc.vector.max_index(top_idx[:], top_val[:], qrow[:])
                nc.vector.tensor_copy(idx_sb[:, ib : ib + 1], top_idx[:, 0:1])

            nc.sync.dma_start(out=idx_d[:], in_=idx_sb[:])

    _reduce_extra_waits(nc)
    return nc


def _reduce_extra_waits(nc):
    """Drop transitively-redundant semaphore waits (walrus codegen allows at
    most ONE sync wait per instruction).

    Sound closure over two in-order streams per instruction:
      - issue stream (engine queue): an instruction executes only after every
        earlier instruction on its engine executed, hence their waits held;
      - completion stream (engine for compute, DMA hw queue for DMAs): a
        semaphore floor s >= v implies every incrementer of s up to v
        completed, hence their waits held and earlier same-proc completions
        fired.
    """
    import sys as _sys

    f = nc.m.functions[0]
    insts = [ins for bb in f.blocks for ins in bb.instructions]
    n_ins = len(insts)
    _sys.setrecursionlimit(max(_sys.getrecursionlimit(), 50 * n_ins + 1000))

    def _upd(ins):
        si = ins.sync_info
        if si is None:
            return None
        for up in si.on_update:
            if up.sync_type == "semaphore" and up.update_mode in (
                "sem-inc",
                "sem-add-imm",
            ):
                return up
        return None

    def _waits(ins):
        si = ins.sync_info
        if si is None:
            return []
        return [
            w
            for w in si.on_wait
            if w.sync_type == "semaphore"
            and w.wait_mode == "sem-ge-imm"
            and w.wait_reg is None
        ]

    sem_incs = {}  # sem id -> list of (cum_value, inst_idx)
    prev_comp = [None] * n_ins
    prev_issue = [None] * n_ins
    last_comp, last_issue = {}, {}
    for k, ins in enumerate(insts):
        up = _upd(ins)
        if up is not None:
            lst = sem_incs.setdefault(up.id, [])
            prev = lst[-1][0] if lst else 0
            lst.append((prev + up.update_value, k))
            proc = ("sem", up.id)
        else:
            proc = ("eng", ins.engine.name)
        if proc in last_comp:
            prev_comp[k] = last_comp[proc]
        last_comp[proc] = k
        ekey = ins.engine.name
        if ekey in last_issue:
            prev_issue[k] = last_issue[ekey]
        last_issue[ekey] = k

    def merge(dst, src):
        for s, v in src.items():
            if dst.get(s, -1) < v:
                dst[s] = v

    issue_memo, comp_memo = {}, {}
    IN_PROGRESS = object()

    def issue_known(k):
        got = issue_memo.get(k)
        if got is IN_PROGRESS:
            return {}
        if got is not None:
            return got
        issue_memo[k] = IN_PROGRESS
        out = {}
        if prev_issue[k] is not None:
            merge(out, issue_known(prev_issue[k]))
        for w in _waits(insts[k]):
            if out.get(w.id, -1) < w.wait_value:
                out[w.id] = w.wait_value
            merge(out, floor_closure(w.id, w.wait_value))
        issue_memo[k] = out
        return out

    def completed(k):
        got = comp_memo.get(k)
        if got is IN_PROGRESS:
            return {}
        if got is not None:
            return got
        comp_memo[k] = IN_PROGRESS
        out = {}
        if prev_comp[k] is not None:
            merge(out, completed(prev_comp[k]))
        merge(out, issue_known(k))
        up = _upd(insts[k])
        if up is not None:
            lst = sem_incs[up.id]
            lo, hi = 0, len(lst)
            while lo < hi:
                mid = (lo + hi) // 2
                if lst[mid][1] <= k:
                    lo = mid + 1
                else:
                    hi = mid
            if lo > 0 and out.get(up.id, -1) < lst[lo - 1][0]:
                out[up.id] = lst[lo - 1][0]
        comp_memo[k] = out
        return out

    def floor_closure(semid, v):
        out = {semid: v}
        lst = sem_incs.get(semid, [])
        lo, hi = 0, len(lst)
        while lo < hi:
            mid = (lo + hi) // 2
            if lst[mid][0] <= v:
                lo = mid + 1
            else:
                hi = mid
        if lo > 0:
            merge(out, completed(lst[lo - 1][1]))
        return out

    bad = []
    for k, ins in enumerate(insts):
        si = ins.sync_info
        if si is None or len(si.on_wait) <= 1:
            continue
        waits = list(si.on_wait)
        changed = True
        while len(waits) > 1 and changed:
            changed = False
            for wi, w in enumerate(waits):
                if not (
                    w.sync_type == "semaphore"
                    and w.wait_mode == "sem-ge-imm"
                    and w.wait_reg is None
                ):
                    continue
                known = {}
                if prev_issue[k] is not None:
                    merge(known, issue_known(prev_issue[k]))
                for wj, w2 in enumerate(waits):
                    if wj == wi:
                        continue
                    if (
                        w2.sync_type == "semaphore"
                        and w2.wait_mode == "sem-ge-imm"
                        and w2.wait_reg is None
                    ):
                        if known.get(w2.id, -1) < w2.wait_value:
                            known[w2.id] = w2.wait_value
                        merge(known, floor_closure(w2.id, w2.wait_value))
                if known.get(w.id, -1) >= w.wait_value:
                    waits.pop(wi)
                    changed = True
                    break
        if len(waits) > 1:
            bad.append(
                (ins.name, [(w.ant_name, w.wait_value) for w in waits])
            )
        if len(waits) != len(si.on_wait):
            si.on_wait = waits
            ins.sync_info = si
    if bad:
        raise RuntimeError(
            f"instructions still have >1 sync wait after reduction: "
            f"{bad[:5]} ({len(bad)} total)"
        )


def _get_program(n, m, nb_limit=None):
    key = (n, m, nb_limit)
    if key not in _PROG_CACHE:
        _PROG_CACHE[key] = _build_program(n, m, nb_limit)
    return _PROG_CACHE[key]


def _l2norm(x):
    nrm = np.sqrt((x * x).sum(axis=-1, keepdims=True))
    return x / np.maximum(nrm, EPS)


def _host_inputs(q6, db6, dbsq, n, m):
    ab = np.empty((7, n + m), np.float16)
    ab[0:6, 0:n] = q6.T
    ab[6, 0:n] = 1.0
    ab[0:6, n:] = 2.0 * db6.T
    ab[6, n:] = -dbsq
    return {"ab": ab}


_LAST_RUN_INFO = {}
_RUNNER_CACHE = {}
_POOL = ThreadPoolExecutor(max_workers=8)


def _get_runner(n, m, n_cores):
    """Build (once) a persistent jitted SPMD executor for the program."""
    key = (n, m, n_cores)
    if key in _RUNNER_CACHE:
        return _RUNNER_CACHE[key]

    import jax
    from jax.experimental.shard_map import shard_map
    from jax.sharding import Mesh, NamedSharding, PartitionSpec

    from concourse import bass2jax, mybir

    nc = _get_program(n, m)
    bass2jax.install_neuronx_cc_hook()

    partition_name = (
        nc.partition_id_tensor.name if nc.partition_id_tensor else None
    )
    in_names, out_names, out_avals, zero_outs = [], [], [], []
    for alloc in nc.m.functions[0].allocations:
        if not isinstance(alloc, mybir.MemoryLocationSet):
            continue
        name = alloc.memorylocations[0].name
        if alloc.kind == "ExternalInput":
            if name != partition_name:
                in_names.append(name)
        elif alloc.kind == "ExternalOutput":
            out_names.append(name)
            shape = tuple(alloc.tensor_shape)
            dtype = mybir.dt.np(alloc.dtype)
            out_avals.append(jax.core.ShapedArray(shape, dtype))
            zero_outs.append(np.zeros(shape, dtype))
    n_params = len(in_names)
    n_outs = len(out_avals)
    in_names_all = list(in_names) + list(out_names)
    if partition_name is not None:
        in_names_all.append(partition_name)

    def _body(*args):
        operands = list(args)
        if partition_name is not None:
            operands.append(bass2jax.partition_id_tensor())
        outs = bass2jax._bass_exec_p.bind(
            *operands,
            out_avals=tuple(out_avals),
            in_names=tuple(in_names_all),
            out_names=tuple(out_names),
            lowering_input_output_aliases=(),
            sim_require_finite=True,
            sim_require_nnan=True,
            nc=nc,
        )
        return tuple(outs)

    donate = tuple(range(n_params, n_params + n_outs))
    devices = jax.devices()[:n_cores]
    mesh = Mesh(np.asarray(devices), ("core",))
    sharded = jax.jit(
        shard_map(
            _body,
            mesh=mesh,
            in_specs=(PartitionSpec("core"),) * (n_params + n_outs),
            out_specs=(PartitionSpec("core"),) * n_outs,
            check_rep=False,
        ),
        donate_argnums=donate,
        keep_unused=True,
    )

    runner = {
        "sharded": sharded,
        "in_names": in_names,
        "out_names": out_names,
        "out_avals": out_avals,
        "zero_outs": zero_outs,
        "n_cores": n_cores,
        "devices": devices,
        "sharding": NamedSharding(mesh, PartitionSpec("core")),
        "prev_outs": None,  # device-resident donation buffers
    }
    _RUNNER_CACHE[key] = runner
    return runner


def _run_jobs(in_maps, n, m):
    import time

    import jax

    n_cores = len(in_maps)
    r = _get_runner(n, m, n_cores)
    devices = r["devices"]

    t0 = time.time()
    # threaded per-shard upload (concurrent RPCs overlap through the tunnel);
    # in_maps entries may be callables so host-side input packing overlaps
    # with the uploads too
    def put(c):
        im = in_maps[c]() if callable(in_maps[c]) else in_maps[c]
        return [
            jax.device_put(im[name], devices[c]) for name in r["in_names"]
        ]

    shard_lists = list(_POOL.map(put, range(n_cores)))
    global_ins = []
    for i, name in enumerate(r["in_names"]):
        shards = [shard_lists[c][i] for c in range(n_cores)]
        aval0 = shards[0].shape
        gshape = (n_cores * aval0[0],) + tuple(aval0[1:])
        global_ins.append(
            jax.make_array_from_single_device_arrays(
                gshape, r["sharding"], shards
            )
        )

    if r["prev_outs"] is None:
        donation = [
            jax.device_put(
                np.zeros((n_cores * z.shape[0], *z.shape[1:]), z.dtype),
                r["sharding"],
            )
            for z in r["zero_outs"]
        ]
    else:
        donation = r["prev_outs"]

    out_arrs = r["sharded"](*global_ins, *donation)
    out_arrs = list(out_arrs)
    r["prev_outs"] = out_arrs

    # threaded per-shard fetch
    def fetch(shard):
        return np.asarray(shard.data)

    outs_np = []
    for a in out_arrs:
        shards = sorted(
            a.addressable_shards, key=lambda s: s.device.id
        )
        parts = list(_POOL.map(fetch, shards))
        outs_np.append(parts)

    _LAST_RUN_INFO["exec_wall_ns"] = (time.time() - t0) * 1e9
    _LAST_RUN_INFO["exec_time_ns"] = None
    name_i = {name: i for i, name in enumerate(r["out_names"])}
    return outs_np[name_i["idx"]]


def kernel(xyz1, xyz2, normal_rebuild, normal_gt):
    xyz1 = np.asarray(xyz1, np.float32)
    xyz2 = np.asarray(xyz2, np.float32)
    normal_rebuild = np.asarray(normal_rebuild, np.float32)
    normal_gt = np.asarray(normal_gt, np.float32)
    b, n = xyz1.shape[0], xyz1.shape[1]
    m = xyz2.shape[1]

    n1 = _l2norm(normal_rebuild)
    n2 = _l2norm(normal_gt)
    p1 = np.concatenate([xyz1, n1], axis=2)
    p2 = np.concatenate([xyz2, n2], axis=2)
    sq1 = (p1 * p1).sum(axis=2)
    sq2 = (p2 * p2).sum(axis=2)

    in_maps = []
    for core in range(2 * b):
        bi, d = core // 2, core % 2
        if d == 0:
            in_maps.append(
                lambda bi=bi: _host_inputs(p1[bi], p2[bi], sq2[bi], n, m)
            )
        else:
            in_maps.append(
                lambda bi=bi: _host_inputs(p2[bi], p1[bi], sq1[bi], n, m)
            )

    outs = _run_jobs(in_maps, n, m)

    xyz_sums = [0.0, 0.0]
    nrm_sums = [0.0, 0.0]
    counts = [0, 0]
    for core, idx in enumerate(outs):
        bi, d = core // 2, core % 2
        if d == 0:
            q_xyz, q_n = xyz1[bi], n1[bi]
            db_xyz, db_n = xyz2[bi], n2[bi]
        else:
            q_xyz, q_n = xyz2[bi], n2[bi]
            db_xyz, db_n = xyz1[bi], n1[bi]
        # idx[p, ib] = argmin for query row ib*128 + p
        j = idx.T.ravel().astype(np.int64)
        t_xyz = db_xyz[j]
        t_n = db_n[j]
        xyz_d = ((q_xyz - t_xyz) ** 2).sum(axis=1)
        nd = np.minimum(
            ((q_n - t_n) ** 2).sum(axis=1), ((q_n + t_n) ** 2).sum(axis=1)
        )
        xyz_sums[d] += float(xyz_d.sum())
        nrm_sums[d] += float(nd.sum())
        counts[d] += n

    xyz_out = xyz_sums[0] / counts[0] + xyz_sums[1] / counts[1]
    nrm_out = nrm_sums[0] / counts[0] + nrm_sums[1] / counts[1]
    return (np.float32(xyz_out), np.float32(nrm_out))


# revision 5
# speedup vs baseline: 1.3173x; 1.3173x over previous
"""Chamfer-with-normals (6D NN search) Trainium2 kernel, v3.

Device program (per core, SPMD over 8 cores, no collectives):
  - 8 jobs = (batch b in 0..3) x (direction in {1,2}); core = 2*b + dir.
  - q[i,j] = 2*q6_i.db6_j - |db6_j|^2 via K=7 fp16 PE matmuls (f32 psum),
    DVE copy to SBUF, DVE max/max_index -> top-1 db index per query row.
  - Output: [128, 64] uint16 index matrix (16 KB) per core.

Host/runner:
  - ships one fp16 [7, n+m] matrix per core; exact fp32 metric on host.
  - per-core shards are device_put from 8 threads (axon RPCs overlap when
    issued concurrently; serial shard uploads dominate the wall otherwise),
    then assembled with make_array_from_single_device_arrays.
  - the donated output buffer is the previous call's device-resident output
    (first call uploads zeros once); output shards are fetched in threads.
"""

import sys
from concurrent.futures import ThreadPoolExecutor

import numpy as np

for _p in ("/opt/trn_rl_repo", "/opt/pypackages"):
    if _p not in sys.path:
        sys.path.insert(0, _p)

B = 4
N = 8192  # queries per job
M = 8192  # database per job
P = 128
EPS = 1e-12

_PROG_CACHE = {}


def _build_program(n, m, nb_limit=None):
    import concourse.bass as bass
    import concourse.tile as tile
    from concourse import mybir

    f16 = mybir.dt.float16
    f32 = mybir.dt.float32
    u16 = mybir.dt.uint16
    nb = nb_limit if nb_limit is not None else n // P  # query row blocks
    K = 7

    nc = bass.Bass()
    ab_d = nc.dram_tensor("ab", [K, n + m], f16, kind="ExternalInput")
    idx_d = nc.dram_tensor("idx", [P, nb], u16, kind="ExternalOutput")

    with tile.TileContext(nc) as tc:
        with (
            tc.tile_pool(name="singles", bufs=1) as singles,
            tc.tile_pool(name="qrows", bufs=2) as qrows,
            tc.tile_pool(name="tops", bufs=4) as tops,
            tc.tile_pool(name="qps", bufs=2, space="PSUM") as qps,
        ):
            ab_sb = singles.tile([K, n + m], f16)
            idx_sb = singles.tile([P, nb], u16)
            nc.sync.dma_start(out=ab_sb[:], in_=ab_d[:])

            for ib in range(nb):
                qrow = qrows.tile([P, m], f32)
                for rnd in range(m // 2048):
                    q = qps.tile([P, 2048], f32, space="PSUM")
                    for r in range(4):
                        c = rnd * 4 + r
                        nc.tensor.matmul(
                            out=q[:, r * 512 : (r + 1) * 512],
                            lhsT=ab_sb[:, ib * P : (ib + 1) * P],
                            rhs=ab_sb[:, n + c * 512 : n + (c + 1) * 512],
                            start=True,
                            stop=True,
                        )
                    nc.vector.tensor_copy(
                        qrow[:, rnd * 2048 : (rnd + 1) * 2048], q[:]
                    )
                top_val = tops.tile([P, 8], f32)
                top_idx = tops.tile([P, 8], u16)
                nc.vector.max(top_val[:], qrow[:])
                nc.vector.max_index(top_idx[:], top_val[:], qrow[:])
                nc.vector.tensor_copy(idx_sb[:, ib : ib + 1], top_idx[:, 0:1])

            nc.sync.dma_start(out=idx_d[:], in_=idx_sb[:])

    _reduce_extra_waits(nc)
    return nc


def _reduce_extra_waits(nc):
    """Drop transitively-redundant semaphore waits (walrus codegen allows at
    most ONE sync wait per instruction).

    Sound closure over two in-order streams per instruction:
      - issue stream (engine queue): an instruction executes only after every
        earlier instruction on its engine executed, hence their waits held;
      - completion stream (engine for compute, DMA hw queue for DMAs): a
        semaphore floor s >= v implies every incrementer of s up to v
        completed, hence their waits held and earlier same-proc completions
        fired.
    """
    import sys as _sys

    f = nc.m.functions[0]
    insts = [ins for bb in f.blocks for ins in bb.instructions]
    n_ins = len(insts)
    _sys.setrecursionlimit(max(_sys.getrecursionlimit(), 50 * n_ins + 1000))

    def _upd(ins):
        si = ins.sync_info
        if si is None:
            return None
        for up in si.on_update:
            if up.sync_type == "semaphore" and up.update_mode in (
                "sem-inc",
                "sem-add-imm",
            ):
                return up
        return None

    def _waits(ins):
        si = ins.sync_info
        if si is None:
            return []
        return [
            w
            for w in si.on_wait
            if w.sync_type == "semaphore"
            and w.wait_mode == "sem-ge-imm"
            and w.wait_reg is None
        ]

    sem_incs = {}  # sem id -> list of (cum_value, inst_idx)
    prev_comp = [None] * n_ins
    prev_issue = [None] * n_ins
    last_comp, last_issue = {}, {}
    for k, ins in enumerate(insts):
        up = _upd(ins)
        if up is not None:
            lst = sem_incs.setdefault(up.id, [])
            prev = lst[-1][0] if lst else 0
            lst.append((prev + up.update_value, k))
            proc = ("sem", up.id)
        else:
            proc = ("eng", ins.engine.name)
        if proc in last_comp:
            prev_comp[k] = last_comp[proc]
        last_comp[proc] = k
        ekey = ins.engine.name
        if ekey in last_issue:
            prev_issue[k] = last_issue[ekey]
        last_issue[ekey] = k

    def merge(dst, src):
        for s, v in src.items():
            if dst.get(s, -1) < v:
                dst[s] = v

    issue_memo, comp_memo = {}, {}
    IN_PROGRESS = object()

    def issue_known(k):
        got = issue_memo.get(k)
        if got is IN_PROGRESS:
            return {}
        if got is not None:
            return got
        issue_memo[k] = IN_PROGRESS
        out = {}
        if prev_issue[k] is not None:
            merge(out, issue_known(prev_issue[k]))
        for w in _waits(insts[k]):
            if out.get(w.id, -1) < w.wait_value:
                out[w.id] = w.wait_value
            merge(out, floor_closure(w.id, w.wait_value))
        issue_memo[k] = out
        return out

    def completed(k):
        got = comp_memo.get(k)
        if got is IN_PROGRESS:
            return {}
        if got is not None:
            return got
        comp_memo[k] = IN_PROGRESS
        out = {}
        if prev_comp[k] is not None:
            merge(out, completed(prev_comp[k]))
        merge(out, issue_known(k))
        up = _upd(insts[k])
        if up is not None:
            lst = sem_incs[up.id]
            lo, hi = 0, len(lst)
            while lo < hi:
                mid = (lo + hi) // 2
                if lst[mid][1] <= k:
                    lo = mid + 1
                else:
                    hi = mid
            if lo > 0 and out.get(up.id, -1) < lst[lo - 1][0]:
                out[up.id] = lst[lo - 1][0]
        comp_memo[k] = out
        return out

    def floor_closure(semid, v):
        out = {semid: v}
        lst = sem_incs.get(semid, [])
        lo, hi = 0, len(lst)
        while lo < hi:
            mid = (lo + hi) // 2
            if lst[mid][0] <= v:
                lo = mid + 1
            else:
                hi = mid
        if lo > 0:
            merge(out, completed(lst[lo - 1][1]))
        return out

    bad = []
    for k, ins in enumerate(insts):
        si = ins.sync_info
        if si is None or len(si.on_wait) <= 1:
            continue
        waits = list(si.on_wait)
        changed = True
        while len(waits) > 1 and changed:
            changed = False
            for wi, w in enumerate(waits):
                if not (
                    w.sync_type == "semaphore"
                    and w.wait_mode == "sem-ge-imm"
                    and w.wait_reg is None
                ):
                    continue
                known = {}
                if prev_issue[k] is not None:
                    merge(known, issue_known(prev_issue[k]))
                for wj, w2 in enumerate(waits):
                    if wj == wi:
                        continue
                    if (
                        w2.sync_type == "semaphore"
                        and w2.wait_mode == "sem-ge-imm"
                        and w2.wait_reg is None
                    ):
                        if known.get(w2.id, -1) < w2.wait_value:
                            known[w2.id] = w2.wait_value
                        merge(known, floor_closure(w2.id, w2.wait_value))
                if known.get(w.id, -1) >= w.wait_value:
                    waits.pop(wi)
                    changed = True
                    break
        if len(waits) > 1:
            bad.append(
                (ins.name, [(w.ant_name, w.wait_value) for w in waits])
            )
        if len(waits) != len(si.on_wait):
            si.on_wait = waits
            ins.sync_info = si
    if bad:
        raise RuntimeError(
            f"instructions still have >1 sync wait after reduction: "
            f"{bad[:5]} ({len(bad)} total)"
        )


def _get_program(n, m, nb_limit=None):
    key = (n, m, nb_limit)
    if key not in _PROG_CACHE:
        _PROG_CACHE[key] = _build_program(n, m, nb_limit)
    return _PROG_CACHE[key]


def _l2norm(x):
    nrm = np.sqrt((x * x).sum(axis=-1, keepdims=True))
    return x / np.maximum(nrm, EPS)


def _host_inputs(q6, db6, dbsq, n, m):
    ab = np.empty((7, n + m), np.float16)
    ab[0:6, 0:n] = q6.T
    ab[6, 0:n] = 1.0
    ab[0:6, n:] = 2.0 * db6.T
    ab[6, n:] = -dbsq
    return {"ab": ab}


_LAST_RUN_INFO = {}
_RUNNER_CACHE = {}
_POOL = ThreadPoolExecutor(max_workers=8)


def _get_runner(n, m, n_cores):
    """Build (once) a persistent jitted SPMD executor for the program."""
    key = (n, m, n_cores)
    if key in _RUNNER_CACHE:
        return _RUNNER_CACHE[key]

    import jax
    from jax.experimental.shard_map import shard_map
    from jax.sharding import Mesh, NamedSharding, PartitionSpec

    from concourse import bass2jax, mybir

    nc = _get_program(n, m)
    bass2jax.install_neuronx_cc_hook()

    partition_name = (
        nc.partition_id_tensor.name if nc.partition_id_tensor else None
    )
    in_names, out_names, out_avals, zero_outs = [], [], [], []
    for alloc in nc.m.functions[0].allocations:
        if not isinstance(alloc, mybir.MemoryLocationSet):
            continue
        name = alloc.memorylocations[0].name
        if alloc.kind == "ExternalInput":
            if name != partition_name:
                in_names.append(name)
        elif alloc.kind == "ExternalOutput":
            out_names.append(name)
            shape = tuple(alloc.tensor_shape)
            dtype = mybir.dt.np(alloc.dtype)
            out_avals.append(jax.core.ShapedArray(shape, dtype))
            zero_outs.append(np.zeros(shape, dtype))
    n_params = len(in_names)
    n_outs = len(out_avals)
    in_names_all = list(in_names) + list(out_names)
    if partition_name is not None:
        in_names_all.append(partition_name)

    def _body(*args):
        operands = list(args)
        if partition_name is not None:
            operands.append(bass2jax.partition_id_tensor())
        outs = bass2jax._bass_exec_p.bind(
            *operands,
            out_avals=tuple(out_avals),
            in_names=tuple(in_names_all),
            out_names=tuple(out_names),
            lowering_input_output_aliases=(),
            sim_require_finite=True,
            sim_require_nnan=True,
            nc=nc,
        )
        return tuple(outs)

    donate = tuple(range(n_params, n_params + n_outs))
    devices = jax.devices()[:n_cores]
    mesh = Mesh(np.asarray(devices), ("core",))
    sharded = jax.jit(
        shard_map(
            _body,
            mesh=mesh,
            in_specs=(PartitionSpec("core"),) * (n_params + n_outs),
            out_specs=(PartitionSpec("core"),) * n_outs,
            check_rep=False,
        ),
        donate_argnums=donate,
        keep_unused=True,
    )

    runner = {
        "sharded": sharded,
        "in_names": in_names,
        "out_names": out_names,
        "out_avals": out_avals,
        "zero_outs": zero_outs,
        "n_cores": n_cores,
        "devices": devices,
        "sharding": NamedSharding(mesh, PartitionSpec("core")),
        "prev_outs": None,  # device-resident donation buffers
    }
    _RUNNER_CACHE[key] = runner
    return runner


def _run_jobs(in_maps, n, m):
    import time

    import jax

    n_cores = len(in_maps)
    r = _get_runner(n, m, n_cores)
    devices = r["devices"]

    t0 = time.time()
    # threaded per-shard upload (concurrent RPCs overlap through the tunnel)
    def put(c):
        return [
            jax.device_put(in_maps[c][name], devices[c])
            for name in r["in_names"]
        ]

    shard_lists = list(_POOL.map(put, range(n_cores)))
    global_ins = []
    for i, name in enumerate(r["in_names"]):
        shards = [shard_lists[c][i] for c in range(n_cores)]
        aval0 = shards[0].shape
        gshape = (n_cores * aval0[0],) + tuple(aval0[1:])
        global_ins.append(
            jax.make_array_from_single_device_arrays(
                gshape, r["sharding"], shards
            )
        )

    if r["prev_outs"] is None:
        donation = [
            jax.device_put(
                np.zeros((n_cores * z.shape[0], *z.shape[1:]), z.dtype),
                r["sharding"],
            )
            for z in r["zero_outs"]
        ]
    else:
        donation = r["prev_outs"]

    out_arrs = r["sharded"](*global_ins, *donation)
    out_arrs = list(out_arrs)
    r["prev_outs"] = out_arrs

    # threaded per-shard fetch
    def fetch(shard):
        return np.asarray(shard.data)

    outs_np = []
    for a in out_arrs:
        shards = sorted(
            a.addressable_shards, key=lambda s: s.device.id
        )
        parts = list(_POOL.map(fetch, shards))
        outs_np.append(parts)

    _LAST_RUN_INFO["exec_wall_ns"] = (time.time() - t0) * 1e9
    _LAST_RUN_INFO["exec_time_ns"] = None
    name_i = {name: i for i, name in enumerate(r["out_names"])}
    return outs_np[name_i["idx"]]


def kernel(xyz1, xyz2, normal_rebuild, normal_gt):
    xyz1 = np.asarray(xyz1, np.float32)
    xyz2 = np.asarray(xyz2, np.float32)
    normal_rebuild = np.asarray(normal_rebuild, np.float32)
    normal_gt = np.asarray(normal_gt, np.float32)
    b, n = xyz1.shape[0], xyz1.shape[1]
    m = xyz2.shape[1]

    n1 = _l2norm(normal_rebuild)
    n2 = _l2norm(normal_gt)
    p1 = np.concatenate([xyz1, n1], axis=2)
    p2 = np.concatenate([xyz2, n2], axis=2)
    sq1 = (p1 * p1).sum(axis=2)
    sq2 = (p2 * p2).sum(axis=2)

    in_maps = []
    for core in range(2 * b):
        bi, d = core // 2, core % 2
        if d == 0:
            in_maps.append(_host_inputs(p1[bi], p2[bi], sq2[bi], n, m))
        else:
            in_maps.append(_host_inputs(p2[bi], p1[bi], sq1[bi], n, m))

    outs = _run_jobs(in_maps, n, m)

    xyz_sums = [0.0, 0.0]
    nrm_sums = [0.0, 0.0]
    counts = [0, 0]
    for core, idx in enumerate(outs):
        bi, d = core // 2, core % 2
        if d == 0:
            q_xyz, q_n = xyz1[bi], n1[bi]
            db_xyz, db_n = xyz2[bi], n2[bi]
        else:
            q_xyz, q_n = xyz2[bi], n2[bi]
            db_xyz, db_n = xyz1[bi], n1[bi]
        # idx[p, ib] = argmin for query row ib*128 + p
        j = idx.T.ravel().astype(np.int64)
        t_xyz = db_xyz[j]
        t_n = db_n[j]
        xyz_d = ((q_xyz - t_xyz) ** 2).sum(axis=1)
        nd = np.minimum(
            ((q_n - t_n) ** 2).sum(axis=1), ((q_n + t_n) ** 2).sum(axis=1)
        )
        xyz_sums[d] += float(xyz_d.sum())
        nrm_sums[d] += float(nd.sum())
        counts[d] += n

    xyz_out = xyz_sums[0] / counts[0] + xyz_sums[1] / counts[1]
    nrm_out = nrm_sums[0] / counts[0] + nrm_sums[1] / counts[1]
    return (np.float32(xyz_out), np.float32(nrm_out))
